# revision 1
# baseline (speedup 1.0000x reference)
"""BitNet-style binary linear: y = x @ w_q.T + bias, w_q = clip(round(w/g))*g.

Strategy (8 NeuronCores, tensor-parallel on out_features):
  - Host: g = max(mean|w|, 1e-5); s = clip(rint(w/g), -1, 1). s is ternary so
    it is EXACT in fp8e4. Fold g into x (xs = g*x) so the device matmul needs
    no rescale; the only precision loss is one fp16 rounding of x (~2e-4
    relative), with fp32 PSUM accumulation.
  - Shard s rows (out_features) 8-ways; replicate x. Each core computes
    out[8192, 2048] = xs @ s_shard.T + bias_shard with all of s_shard.T
    resident in SBUF (8 MB fp8) and x streamed in r-blocks.
  - All device inputs are host-packed into the exact SBUF tile layout
    [128 partitions, k-chunk, cols] so every DMA is a single fully
    contiguous copy at full HBM bandwidth.
  - matmul(psum[r128, f512], lhsT=xs_tile[k128, r128], rhs=s_tile[k128, f512])
    accumulated over 32 k-chunks; bias is pre-broadcast across partitions
    once via a K=1 ones-matmul and added during the DVE psum eviction.
  - Pipeline priming: weights arrive in 4 f-quarters; the first 512 rows are
    processed one f-quarter at a time so the in-order PE always has work
    while later quarters stream in.
"""

import numpy as np

B, S, D_IN, D_OUT = 4, 2048, 4096, 16384
N_CORES = 8
R = B * S                 # 8192 rows of x
F = D_OUT // N_CORES      # 2048 features per core
KC = D_IN // 128          # 32 k-chunks
RB = 512                  # steady-state r-block
FT = 512                  # f-tile (one PSUM bank)
NF = F // FT              # 4 f-tiles == wt quarters
NB = (R - 512) // RB      # 15 steady blocks (rows 512..8192)

_CACHE = {}


def _patch_light_exit():
    """Drop the second all-engine barrier in TileContext's exit: sem clears
    run in each engine's own stream and NRT waits for stream completion
    before any re-execution, so the trailing butterfly only adds ~3us."""
    import concourse.tile as tile
    from concourse.vector_clock import ScopedClock

    if getattr(tile.TileContext, "_light_exit", False):
        return

    def _drain_and_barrier(self, tick_clock, wait_clock):
        nc = self.nc
        drain_inst = nc.sync.drain()
        wait_clock.add_sem_waits(
            drain_inst.ins, ScopedClock({None: tick_clock.global_clock})
        )
        nc.all_engine_barrier()
        popped = nc._tile_sem_poison_stack.pop()
        assert popped is self._sem_poison
        nc.clear_and_free_semaphores(list(self.sems.allocated().values()))

    tile.TileContext._drain_and_barrier = _drain_and_barrier
    tile.TileContext._light_exit = True


def _build_nc():
    import concourse.mybir as mybir
    import concourse.tile as tile
    from concourse import bacc

    _patch_light_exit()
    fp16 = mybir.dt.float16
    fp8 = mybir.dt.float8e4
    f32 = mybir.dt.float32

    nc = bacc.Bacc("TRN2", target_bir_lowering=False, debug=False,
                   num_devices=N_CORES)
    xh0 = nc.declare_dram_parameter("xh0", [128, KC * 128], fp16, isOutput=False)
    xh1 = nc.declare_dram_parameter("xh1", [128, KC * 384], fp16, isOutput=False)
    xp = nc.declare_dram_parameter("xp", [NB, 128, KC * RB], fp16, isOutput=False)
    wq = nc.declare_dram_parameter("wq", [NF, 128, KC * FT], fp8, isOutput=False)
    bias = nc.declare_dram_parameter("bias", [1, F], fp16, isOutput=False)
    out = nc.declare_dram_parameter("out", [R, F], f32, isOutput=True)

    with tile.TileContext(nc) as tc:
        with (
            tc.tile_pool(name="wpool", bufs=1) as wpool,
            tc.tile_pool(name="cpool", bufs=1) as cpool,
            tc.tile_pool(name="xpool", bufs=2) as xpool,
            tc.tile_pool(name="opool", bufs=4) as opool,
            tc.tile_pool(name="pspool", bufs=4, space="PSUM") as pspool,
        ):
            # broadcast bias across partitions once: ones[1,128].T @ bias[1,512]
            bias_sb = cpool.tile([1, F], fp16, tag="bias")
            nc.sync.dma_start(bias_sb[:], bias[:, :])
            ones_sb = cpool.tile([1, 128], fp16, tag="ones")
            nc.gpsimd.memset(ones_sb[:], 1.0)
            bias_bc = cpool.tile([128, F], f32, tag="bias_bc")
            for f in range(NF):
                bp = pspool.tile([128, FT], f32)
                nc.tensor.matmul(bp[:], ones_sb[:],
                                 bias_sb[:, f * FT:(f + 1) * FT],
                                 start=True, stop=True)
                nc.vector.tensor_copy(bias_bc[:, f * FT:(f + 1) * FT], bp[:])

            # head DMAs in critical-path order: first x rows, then wt
            # quarters (second x block slotted after the first quarter)
            xh0_t = cpool.tile([128, KC * 128], fp16, tag="xh0")
            nc.sync.dma_start(xh0_t[:], xh0[:, :])
            wt_sb = []
            for q in range(NF):
                t = wpool.tile([128, KC * FT], fp8, tag=f"wq{q}")
                nc.sync.dma_start(t[:], wq[q, :, :])
                wt_sb.append(t)
                if q == 0:
                    xh1_t = cpool.tile([128, KC * 384], fp16, tag="xh1")
                    nc.sync.dma_start(xh1_t[:], xh1[:, :])

            def do_tile(xt_t, rbn, rt, r0, f):
                ps = pspool.tile([128, FT], f32)
                for c in range(KC):
                    nc.tensor.matmul(
                        ps[:],
                        xt_t[:, c * rbn + rt * 128:c * rbn + rt * 128 + 128],
                        wt_sb[f][:, c * FT:(c + 1) * FT],
                        start=(c == 0), stop=(c == KC - 1),
                    )
                ob = opool.tile([128, FT], f32)
                nc.vector.tensor_add(
                    ob[:], ps[:], bias_bc[:, f * FT:(f + 1) * FT]
                )
                nc.sync.dma_start(
                    out[r0:r0 + 128, f * FT:(f + 1) * FT], ob[:]
                )

            # prime: rows 0..512, one f-quarter at a time (PE is in-order;
            # quarter f+1 streams in while quarter f computes)
            for f in range(NF):
                do_tile(xh0_t, 128, 0, 0, f)
                for rt in range(3):
                    do_tile(xh1_t, 384, rt, 128 + rt * 128, f)

            # steady state
            for b in range(NB):
                xt_t = xpool.tile([128, KC * RB], fp16)
                nc.sync.dma_start(xt_t[:], xp[b, :, :])
                for rt in range(RB // 128):
                    for f in range(NF):
                        do_tile(xt_t, RB, rt, 512 + b * RB + rt * 128, f)
    nc.compile()
    return nc


def _pack(a):
    """[rows, D_IN] -> [128, KC*rows] in SBUF layout (partition = k%128)."""
    rows = a.shape[0]
    return np.ascontiguousarray(
        a.T.reshape(KC, 128, rows).transpose(1, 0, 2)
    ).reshape(128, KC * rows)


def _prepare_in_maps(x, weight, bias):
    import ml_dtypes

    x = np.asarray(x)
    weight = np.asarray(weight)
    bias = np.asarray(bias)

    gamma = np.float32(max(np.mean(np.abs(weight), dtype=np.float64), 1e-5))
    s = np.clip(np.rint(weight.astype(np.float32) / gamma), -1.0, 1.0)

    xs = (x.reshape(R, D_IN) * gamma).astype(np.float16)
    xh0 = _pack(xs[0:128])
    xh1 = _pack(xs[128:512])
    xp = np.stack([_pack(xs[512 + b * RB:512 + (b + 1) * RB]) for b in range(NB)])

    b16 = bias.astype(np.float16)
    in_maps = []
    for c in range(N_CORES):
        sh = s[c * F:(c + 1) * F].astype(ml_dtypes.float8_e4m3)  # [F, D_IN]
        wqq = np.stack([_pack(sh[q * FT:(q + 1) * FT, :]) for q in range(NF)])
        in_maps.append({
            "xh0": xh0, "xh1": xh1, "xp": xp, "wq": wqq,
            "bias": np.ascontiguousarray(b16[c * F:(c + 1) * F]).reshape(1, F),
        })
    return in_maps


def _assemble(results):
    out = np.concatenate([results[c]["out"] for c in range(N_CORES)], axis=1)
    return out.reshape(B, S, D_OUT)


def kernel(x, weight, bias):
    import os
    import time
    os.environ.setdefault("BASS_NEVER_TRACE", "1")
    from concourse.bass_utils import run_bass_kernel_spmd

    in_maps = _prepare_in_maps(x, weight, bias)
    if "nc" not in _CACHE:
        _CACHE["nc"] = _build_nc()
    last_err = None
    for attempt in range(3):
        try:
            res = run_bass_kernel_spmd(
                _CACHE["nc"], in_maps, core_ids=list(range(N_CORES)))
            return _assemble(res.results)
        except Exception as e:  # transient device errors (e.g. prior process
            last_err = e        # still tearing down) clear after ~30s
            time.sleep(30 * (attempt + 1))
    raise last_err



# revision 2
# speedup vs baseline: 1.3193x; 1.3193x over previous
"""BitNet-style binary linear: y = x @ w_q.T + bias, w_q = clip(round(w/g))*g.

Strategy (8 NeuronCores, tensor-parallel on out_features):
  - Host: g = max(mean|w|, 1e-5); s = clip(rint(w/g), -1, 1). s is ternary so
    it is EXACT in fp8e4. x is scaled by SC=32 and split along K: the first
    N16=16 k-chunks are fp16 (exact to ~2e-4), the remaining 16 chunks are
    fp8e4m3 and computed pairwise with perf_mode=DoubleRow at 2x PE rate
    (measured 228 ns per K=256/N=512 slab vs 438 ns at the 16-bit rate).
    End-to-end l2 relative error 1.88e-2 (gate 2e-2), deterministic.
  - Shard s rows (out_features) 8-ways; replicate x. Each core computes
    out[8192, 2048] = xs @ s_shard.T with all of s_shard.T resident in SBUF
    (8 MB fp8) and x streamed in r-blocks (fp16 half + fp8 half).
  - PSUM eviction fuses the gamma/SC rescale and the bias add in one DVE
    scalar_tensor_tensor: out = (psum * gs) + bias_bc.
  - Host packs every tensor into the exact SBUF tile layout
    [128 partitions, k-chunk, cols] so every DMA is fully contiguous.
  - Pipeline priming: weights arrive in 4 f-quarters; the first 512 rows are
    processed one f-quarter at a time so the in-order PE always has work
    while later quarters stream in.
"""

import numpy as np

B, S, D_IN, D_OUT = 4, 2048, 4096, 16384
N_CORES = 8
R = B * S                 # 8192 rows of x
F = D_OUT // N_CORES      # 2048 features per core
KC = D_IN // 128          # 32 k-chunks
N16 = 16                  # k-chunks computed in fp16 (chunks 0..N16-1)
KC8 = KC - N16            # k-chunks computed in fp8 DoubleRow pairs
SC = 32.0                 # x pre-scale (power of 2; undone at eviction)
RB = 512                  # steady-state r-block
FT = 512                  # f-tile (one PSUM bank)
NF = F // FT              # 4 f-tiles == wt quarters
NB = (R - 512) // RB      # 15 steady blocks (rows 512..8192)

_CACHE = {}


def _patch_light_exit():
    """Drop the second all-engine barrier in TileContext's exit: sem clears
    run in each engine's own stream and NRT waits for stream completion
    before any re-execution, so the trailing butterfly only adds ~3us."""
    import concourse.tile as tile
    from concourse.vector_clock import ScopedClock

    if getattr(tile.TileContext, "_light_exit", False):
        return

    def _drain_and_barrier(self, tick_clock, wait_clock):
        nc = self.nc
        drain_inst = nc.sync.drain()
        wait_clock.add_sem_waits(
            drain_inst.ins, ScopedClock({None: tick_clock.global_clock})
        )
        nc.all_engine_barrier()
        popped = nc._tile_sem_poison_stack.pop()
        assert popped is self._sem_poison
        nc.clear_and_free_semaphores(list(self.sems.allocated().values()))

    tile.TileContext._drain_and_barrier = _drain_and_barrier
    tile.TileContext._light_exit = True


def _build_nc():
    import concourse.mybir as mybir
    import concourse.tile as tile
    from concourse import bacc

    _patch_light_exit()
    fp16 = mybir.dt.float16
    fp8 = mybir.dt.float8e4
    f32 = mybir.dt.float32
    DR = mybir.MatmulPerfMode.DoubleRow
    MULT, ADD = mybir.AluOpType.mult, mybir.AluOpType.add

    nc = bacc.Bacc("TRN2", target_bir_lowering=False, debug=False,
                   num_devices=N_CORES)
    xh16_0 = nc.declare_dram_parameter("xh16_0", [128, N16, 128], fp16, isOutput=False)
    xh16_1 = nc.declare_dram_parameter("xh16_1", [128, N16, 384], fp16, isOutput=False)
    xp16 = nc.declare_dram_parameter("xp16", [NB, 128, N16, RB], fp16, isOutput=False)
    xh8_0 = nc.declare_dram_parameter("xh8_0", [128, KC8, 128], fp8, isOutput=False)
    xh8_1 = nc.declare_dram_parameter("xh8_1", [128, KC8, 384], fp8, isOutput=False)
    xp8 = nc.declare_dram_parameter("xp8", [NB, 128, KC8, RB], fp8, isOutput=False)
    wq = nc.declare_dram_parameter("wq", [NF, 128, KC, FT], fp8, isOutput=False)
    bias = nc.declare_dram_parameter("bias", [1, F], fp16, isOutput=False)
    gs = nc.declare_dram_parameter("gs", [128, 1], f32, isOutput=False)
    out = nc.declare_dram_parameter("out", [R, F], f32, isOutput=True)

    with tile.TileContext(nc) as tc:
        with (
            tc.tile_pool(name="wpool", bufs=1) as wpool,
            tc.tile_pool(name="cpool", bufs=1) as cpool,
            tc.tile_pool(name="xpool", bufs=2) as xpool,
            tc.tile_pool(name="opool", bufs=4) as opool,
            tc.tile_pool(name="pspool", bufs=4, space="PSUM") as pspool,
        ):
            # broadcast bias across partitions once: ones[1,128].T @ bias[1,512]
            bias_sb = cpool.tile([1, F], fp16, tag="bias")
            nc.sync.dma_start(bias_sb[:], bias[:, :])
            gs_sb = cpool.tile([128, 1], f32, tag="gs")
            nc.sync.dma_start(gs_sb[:], gs[:, :])
            ones_sb = cpool.tile([1, 128], fp16, tag="ones")
            nc.gpsimd.memset(ones_sb[:], 1.0)
            bias_bc = cpool.tile([128, F], f32, tag="bias_bc")
            for f in range(NF):
                bp = pspool.tile([128, FT], f32)
                nc.tensor.matmul(bp[:], ones_sb[:],
                                 bias_sb[:, f * FT:(f + 1) * FT],
                                 start=True, stop=True)
                nc.vector.tensor_copy(bias_bc[:, f * FT:(f + 1) * FT], bp[:])

            # head DMAs in critical-path order: first x rows, then wt
            # quarters (second x block slotted after the first quarter)
            xh16_0t = cpool.tile([128, N16, 128], fp16, tag="xh16_0")
            nc.sync.dma_start(xh16_0t[:], xh16_0[:, :, :])
            xh8_0t = cpool.tile([128, KC8, 128], fp8, tag="xh8_0")
            nc.sync.dma_start(xh8_0t[:], xh8_0[:, :, :])
            wt_sb = []
            for q in range(NF):
                t = wpool.tile([128, KC, FT], fp8, tag=f"wq{q}")
                nc.sync.dma_start(t[:], wq[q, :, :, :])
                wt_sb.append(t)
                if q == 0:
                    xh16_1t = cpool.tile([128, N16, 384], fp16, tag="xh16_1")
                    nc.sync.dma_start(xh16_1t[:], xh16_1[:, :, :])
                    xh8_1t = cpool.tile([128, KC8, 384], fp8, tag="xh8_1")
                    nc.sync.dma_start(xh8_1t[:], xh8_1[:, :, :])

            def do_tile(xt16, xt8, rt, r0, f):
                wt = wt_sb[f]
                ps = pspool.tile([128, FT], f32)
                c0 = rt * 128
                for c in range(N16):
                    nc.tensor.matmul(
                        ps[:],
                        xt16[:, c, c0:c0 + 128],
                        wt[:, c, :],
                        start=(c == 0), stop=False,
                    )
                for j in range(KC8 // 2):
                    nc.tensor.matmul(
                        ps[:],
                        xt8[:, 2 * j:2 * j + 2, c0:c0 + 128],
                        wt[:, N16 + 2 * j:N16 + 2 * j + 2, :],
                        start=False, stop=(j == KC8 // 2 - 1),
                        perf_mode=DR,
                    )
                ob = opool.tile([128, FT], f32)
                nc.vector.scalar_tensor_tensor(
                    ob[:], ps[:], gs_sb[:, 0:1],
                    bias_bc[:, f * FT:(f + 1) * FT],
                    op0=MULT, op1=ADD,
                )
                nc.sync.dma_start(
                    out[r0:r0 + 128, f * FT:(f + 1) * FT], ob[:]
                )

            # prime: rows 0..512, one f-quarter at a time (PE is in-order;
            # quarter f+1 streams in while quarter f computes)
            for f in range(NF):
                do_tile(xh16_0t, xh8_0t, 0, 0, f)
                for rt in range(3):
                    do_tile(xh16_1t, xh8_1t, rt, 128 + rt * 128, f)

            # steady state
            for b in range(NB):
                xt16 = xpool.tile([128, N16, RB], fp16)
                nc.sync.dma_start(xt16[:], xp16[b, :, :, :])
                xt8 = xpool.tile([128, KC8, RB], fp8)
                nc.sync.dma_start(xt8[:], xp8[b, :, :, :])
                for rt in range(RB // 128):
                    for f in range(NF):
                        do_tile(xt16, xt8, rt, 512 + b * RB + rt * 128, f)
    nc.compile()
    return nc


def _pack(a):
    """[rows, k] -> [128, k//128, rows] in SBUF layout (partition = k%128)."""
    rows = a.shape[0]
    kc = a.shape[1] // 128
    return np.ascontiguousarray(a.T.reshape(kc, 128, rows).transpose(1, 0, 2))


def _prepare_in_maps(x, weight, bias):
    import ml_dtypes

    x = np.asarray(x)
    weight = np.asarray(weight)
    bias = np.asarray(bias)

    gamma = np.float32(max(np.mean(np.abs(weight), dtype=np.float64), 1e-5))
    s = np.clip(np.rint(weight.astype(np.float32) / gamma), -1.0, 1.0)

    xs = x.reshape(R, D_IN) * np.float32(SC)
    k16 = N16 * 128
    xs16 = xs[:, :k16].astype(np.float16)
    xs8 = xs[:, k16:].astype(ml_dtypes.float8_e4m3)

    xh16_0 = _pack(xs16[0:128])
    xh16_1 = _pack(xs16[128:512])
    xp16 = np.stack([_pack(xs16[512 + b * RB:512 + (b + 1) * RB])
                     for b in range(NB)])
    xh8_0 = _pack(xs8[0:128])
    xh8_1 = _pack(xs8[128:512])
    xp8 = np.stack([_pack(xs8[512 + b * RB:512 + (b + 1) * RB])
                    for b in range(NB)])

    gs = np.full((128, 1), gamma / np.float32(SC), dtype=np.float32)
    b16 = bias.astype(np.float16)
    in_maps = []
    for c in range(N_CORES):
        sh = s[c * F:(c + 1) * F].astype(ml_dtypes.float8_e4m3)  # [F, D_IN]
        wqq = np.stack([_pack(sh[q * FT:(q + 1) * FT, :]) for q in range(NF)])
        in_maps.append({
            "xh16_0": xh16_0, "xh16_1": xh16_1, "xp16": xp16,
            "xh8_0": xh8_0, "xh8_1": xh8_1, "xp8": xp8,
            "wq": wqq, "gs": gs,
            "bias": np.ascontiguousarray(b16[c * F:(c + 1) * F]).reshape(1, F),
        })
    return in_maps


def _assemble(results):
    out = np.concatenate([results[c]["out"] for c in range(N_CORES)], axis=1)
    return out.reshape(B, S, D_OUT)


def kernel(x, weight, bias):
    import os
    import time
    os.environ.setdefault("BASS_NEVER_TRACE", "1")
    from concourse.bass_utils import run_bass_kernel_spmd

    in_maps = _prepare_in_maps(x, weight, bias)
    if "nc" not in _CACHE:
        _CACHE["nc"] = _build_nc()
    last_err = None
    for attempt in range(3):
        try:
            res = run_bass_kernel_spmd(
                _CACHE["nc"], in_maps, core_ids=list(range(N_CORES)))
            return _assemble(res.results)
        except Exception as e:  # transient device errors (e.g. prior process
            last_err = e        # still tearing down) clear after ~30s
            time.sleep(30 * (attempt + 1))
    raise last_err


if __name__ == "__main__":
    import jax
    jax.config.update("jax_platforms", "cpu")
    import reference

    inputs = reference.setup_inputs()
    expected = np.asarray(reference.reference(**inputs))
    actual = kernel(**{k: np.asarray(v) for k, v in inputs.items()})
    err = actual.astype(np.float64) - expected.astype(np.float64)
    l2 = np.sqrt((err ** 2).mean()) / np.sqrt(
        (expected.astype(np.float64) ** 2).mean())
    print(f"Relative error: {l2:.6e}")
